# revision 5
# baseline (speedup 1.0000x reference)
"""BinaryConv (XNOR-style binary-weight 3x3 conv) on 8 Trainium2 NeuronCores.

Full-input contract: kernel(x=[32,256,56,56] f32, weight=[256,256,3,3] f32)
-> [32,256,56,56] f32.

Strategy: data-parallel over batch (4 images/core), weight replicated.
Per core, a 1-D Winograd F(4,3) decomposition along W cuts the tensor-engine
MAC count 2x vs direct convolution (6 transformed positions per 4 output
columns instead of 12 tap-MACs):

  y = A'^T [ (G' s) . (B^T d) ]   per output row, with the 3 kh taps and the
                                  2 ci chunks accumulated in PSUM.

All math on device; host marshalling is layout/dtype only: x ships bf16,
de-interleaved into the six B^T operand planes d0..d5 (stride-4 phases of the
zero-padded rows, flattened 58x15 with a garbage 15th column) so every DVE op
and matmul rhs reads a contiguous, 4B-aligned window.  The weight transform
G'·sign(w) uses the row-rescaled dyadic G' (rows x[1,3,3,12,12,1]) so U is
exact in bf16; the matching column scales 1/c_p fold into the fp32
per-output-channel a=mean|w| (reduced on GPSIMD) applied at PSUM eviction on
ACT.  The input transform B^T and inverse transform A'^T run on DVE in bf16
via fused scalar_tensor_tensor ops; image n+1's input transform is emitted
ahead of image n's inverse transform so the DVE never stalls the PE.  The
output ships phase-major bf16 and is re-interleaved (and cast f32) on host.
"""

import ml_dtypes
import numpy as np

import concourse.mybir as mybir
import concourse.tile as tile
from concourse import bacc
from concourse.bass_utils import run_bass_kernel_spmd

F32 = mybir.dt.float32
BF16 = mybir.dt.bfloat16
ALU = mybir.AluOpType

N_CORES = 8
B, C, H, W = 32, 256, 56, 56
O, KH, KW = 256, 3, 3
BP = B // N_CORES            # images per core
P = 128                      # partitions
NCI = C // P                 # input-channel chunks
NCO = O // P                 # output-channel chunks
NP = 6                       # winograd positions (F(4,3): m+r-1 = 6)
TX = 14                      # output tiles along W (4 cols each)
PR = H + 2                   # padded rows (h -1..56)
PW15 = 15                    # tile columns incl. garbage col 14
DPL = PR * PW15 + 2          # 872: d-plane stride (870 valid + 2 pad)
FLAT = PR * PW15             # 870: flat transform window
OUTF = H * PW15              # 840: output flat length per (co, img)
FB = OUTF // 2               # 420: psum free size (2 batches)
KIN = C * KH * KW            # 2304 per-filter fan-in


def _u_off(p: int, kh: int, ci: int, co: int) -> int:
    return (((p * KH + kh) * NCI + ci) * NCO + co) * P


def build(bp: int = BP):
    nc = bacc.Bacc(
        "TRN2",
        target_bir_lowering=False,
        debug=False,
        enable_asserts=False,
        num_devices=N_CORES,
        enable_partition_id=False,
    )
    x_d = nc.dram_tensor("x", [bp, C, NP, DPL], BF16, kind="ExternalInput")
    w_d = nc.dram_tensor("w", [O, C, KH, KW], F32, kind="ExternalInput")
    wp_d = nc.dram_tensor("wp", [KH * KW, C, O], BF16, kind="ExternalInput")
    out_d = nc.dram_tensor("out", [bp, O, 4, OUTF], BF16,
                           kind="ExternalOutput")

    x = x_d.ap().rearrange("n (c p) s f -> n p c s f", p=P)
    w = w_d.ap().rearrange("o i kh kw -> o (i kh kw)")
    wp = wp_d.ap().rearrange("t (c p) o -> p c t o", p=P)
    out = out_d.ap()

    with tile.TileContext(nc) as tc:
        with (
            tc.tile_pool(name="const", bufs=1) as const_pool,
            tc.tile_pool(name="wstage", bufs=2) as wstage_pool,
            tc.tile_pool(name="xph", bufs=4) as xph_pool,
            tc.tile_pool(name="vt", bufs=4) as v_pool,
            tc.tile_pool(name="tmp", bufs=10) as tmp_pool,
            tc.tile_pool(name="ev", bufs=28) as e_pool,
            tc.tile_pool(name="yt", bufs=14) as y_pool,
            tc.tile_pool(name="psum", bufs=7, space="PSUM") as psum_pool,
            tc.tile_pool(name="warmps", bufs=1, space="PSUM") as warmps_pool,
        ):
            # ---- PE warmup: hold HAM at 2.4GHz through the input ramp ----
            warm_l = const_pool.tile([P, P], BF16)
            warm_r = const_pool.tile([P, 512], BF16)
            nc.gpsimd.memset(warm_l[:], 0.0)
            nc.gpsimd.memset(warm_r[:], 0.0)
            zbias = const_pool.tile([P, 1], F32)
            zscr = const_pool.tile([P, 1], F32)
            nc.gpsimd.memset(zbias[:], 0.0)
            warm_ps = warmps_pool.tile([P, 512], F32)
            for _ in range(14):
                nc.tensor.matmul(warm_ps[:], warm_l[:], warm_r[:],
                                 start=True, stop=True)
            # preload the Sign LUT on ACT before the weights arrive
            nc.scalar.sign(zscr[:], zbias[:], bias=zbias[:])

            # ---- critical-path input DMAs on the sync ring (FIFO) --------
            wpt = [const_pool.tile([P, KH * KW, O], BF16, name="wpt")
                   for _ in range(NCI)]
            nc.sync.dma_start(wpt[0][:], wp[:, 0])
            nc.sync.dma_start(wpt[1][:], wp[:, 1])
            xts: list[list] = [[None] * NCI for _ in range(bp)]

            def emit_x_dma(n, ci):
                xt = xph_pool.tile([P, NP, DPL], BF16, name="xt")
                nc.sync.dma_start(xt[:], x[n, :, ci])
                xts[n][ci] = xt

            emit_x_dma(0, 0)
            emit_x_dma(0, 1)
            wstages = [wstage_pool.tile([P, KIN], F32, name="ws")
                       for _ in range(NCO)]
            nc.sync.dma_start(wstages[0][:], w[0:P, :])
            nc.sync.dma_start(wstages[1][:], w[P:2 * P, :])
            for n in range(1, bp):
                emit_x_dma(n, 0)
                emit_x_dma(n, 1)

            # ---- signs (ACT); U tiles built on DVE, interleaved ----------
            st = [const_pool.tile([P, KH * KW, O], BF16, name="st")
                  for _ in range(NCI)]
            for ci in range(NCI):
                nc.scalar.sign(st[ci][:], wpt[ci][:], bias=zbias[:])

            upack = const_pool.tile([P, NP * KH * NCI * O], BF16)
            up = upack[:].rearrange("q (p kh ci o) -> q p kh ci o",
                                    p=NP, kh=KH, ci=NCI)

            def emit_u0(ci):
                sv = st[ci][:].rearrange("q (kh kw) o -> q kh kw o", kh=KH)
                nc.vector.tensor_scalar_mul(up[:, 0, :, ci], sv[:, :, 0],
                                            0.25)

            def emit_u_rest(ci):
                sv = st[ci][:].rearrange("q (kh kw) o -> q kh kw o", kh=KH)
                s0, s1, s2 = sv[:, :, 0], sv[:, :, 1], sv[:, :, 2]
                u = [up[:, p, :, ci] for p in range(NP)]
                vv = nc.vector
                t1 = const_pool.tile([P, KH, O], BF16, name="t1")
                s1h = const_pool.tile([P, KH, O], BF16, name="s1h")
                q_ = const_pool.tile([P, KH, O], BF16, name="q_")
                vv.tensor_tensor(t1[:], s0, s2, op=ALU.add)
                vv.tensor_scalar_mul(s1h[:], s1, 0.5)
                vv.scalar_tensor_tensor(u[1], t1[:], -0.5, s1h[:],
                                        op0=ALU.mult, op1=ALU.subtract)
                vv.tensor_tensor(u[2], u[1], s1, op=ALU.add)
                vv.scalar_tensor_tensor(q_[:], s2, 2.0, s1,
                                        op0=ALU.mult, op1=ALU.add)
                vv.scalar_tensor_tensor(u[3], s0, 0.5, q_[:],
                                        op0=ALU.mult, op1=ALU.add)
                vv.scalar_tensor_tensor(u[4], s1, -2.0, u[3],
                                        op0=ALU.mult, op1=ALU.add)
                vv.tensor_copy(u[5], s2)

            # ---- a = mean|w| with A'^T column scales (ACT+GPSIMD, off DVE)
            # av[c][:, co]: c=0 -> a, c=1 -> a/3, c=2 -> a/12  (fp32)
            av = [const_pool.tile([P, NCO], F32, name=f"av{c}")
                  for c in range(3)]
            for co in range(NCO):
                asum = wstage_pool.tile([P, 1], F32, name="asum", bufs=2)
                wabs = wstage_pool.tile([P, KIN], F32, name="wabs", bufs=1)
                nc.scalar.activation(wabs[:], wstages[co][:],
                                     mybir.ActivationFunctionType.Abs,
                                     accum_out=asum[:])
                nc.gpsimd.tensor_scalar_mul(av[0][:, co:co + 1], asum[:],
                                            1.0 / KIN)
                nc.gpsimd.tensor_scalar_mul(av[1][:, co:co + 1], asum[:],
                                            1.0 / (3 * KIN))
                nc.gpsimd.tensor_scalar_mul(av[2][:, co:co + 1], asum[:],
                                            1.0 / (12 * KIN))
            av_of_p = [av[0], av[1], av[1], av[2], av[2], av[0]]

            def emit_transform(n, ci):
                """Input transform (DVE): V[p] = B^T d, bf16 contiguous."""
                xt = xts[n][ci]
                d = [xt[:, s, 0:FLAT] for s in range(NP)]
                vt = v_pool.tile([P, NP, FLAT], BF16, name="vt")
                v = [vt[:, p, :] for p in range(NP)]

                def tmp():
                    return tmp_pool.tile([P, FLAT], BF16, name="tw")

                i_, k_, g_, n_ = tmp(), tmp(), tmp(), tmp()
                ap_, bp_, am_, f_ = tmp(), tmp(), tmp(), tmp()
                vv = nc.vector
                vv.tensor_tensor(i_[:], d[4], d[2], op=ALU.subtract)
                vv.tensor_tensor(k_[:], d[0], d[2], op=ALU.subtract)
                vv.scalar_tensor_tensor(v[0], k_[:], 4.0, i_[:],
                                        op0=ALU.mult, op1=ALU.add)
                vv.tensor_tensor(ap_[:], d[1], d[2], op=ALU.add)
                vv.tensor_tensor(bp_[:], d[3], d[4], op=ALU.add)
                vv.scalar_tensor_tensor(v[1], ap_[:], -4.0, bp_[:],
                                        op0=ALU.mult, op1=ALU.add)
                vv.tensor_tensor(am_[:], d[1], d[2], op=ALU.subtract)
                vv.tensor_tensor(f_[:], d[3], d[4], op=ALU.subtract)
                vv.scalar_tensor_tensor(v[2], am_[:], 4.0, f_[:],
                                        op0=ALU.mult, op1=ALU.subtract)
                vv.tensor_tensor(g_[:], d[3], d[1], op=ALU.subtract)
                vv.scalar_tensor_tensor(v[3], g_[:], 2.0, i_[:],
                                        op0=ALU.mult, op1=ALU.add)
                vv.scalar_tensor_tensor(v[4], g_[:], -2.0, i_[:],
                                        op0=ALU.mult, op1=ALU.add)
                vv.tensor_tensor(n_[:], d[5], d[3], op=ALU.subtract)
                vv.scalar_tensor_tensor(v[5], g_[:], -4.0, n_[:],
                                        op0=ALU.mult, op1=ALU.add)
                return vt

            def emit_mms(n, vts):
                """Matmuls + scaled PSUM eviction for image n.

                Batch-paired so the two MMs sharing a stationary U tile are
                adjacent (halves exposed LDWEIGHTS).  Returns E planes
                ev[co][b2][p]."""
                ev = [[[None] * NP for _ in range(2)] for _ in range(NCO)]
                for co in range(NCO):
                    for p in range(NP):
                        ps = [psum_pool.tile([P, FB], F32, name="ps")
                              for _ in range(2)]
                        for ci in range(NCI):
                            for kh in range(KH):
                                off = _u_off(p, kh, ci, co)
                                first = ci == 0 and kh == 0
                                last = ci == NCI - 1 and kh == KH - 1
                                for b2 in range(2):
                                    rhs = vts[ci][:, p,
                                                  PW15 * kh + FB * b2:
                                                  PW15 * kh + FB * b2 + FB]
                                    nc.tensor.matmul(
                                        ps[b2][:], upack[:, off:off + P],
                                        rhs, start=first, stop=last,
                                    )
                        for b2 in range(2):
                            et = e_pool.tile([P, FB], BF16, name="et")
                            nc.scalar.mul(et[:], ps[b2][:],
                                          av_of_p[p][:, co:co + 1])
                            ev[co][b2][p] = et
                return ev

            def emit_y(n, ev):
                """Inverse transform A'^T (DVE, bf16) + output DMA (ACT)."""
                for co in range(NCO):
                    for b2 in range(2):
                        e = ev[co][b2]

                        def yt():
                            return y_pool.tile([P, FB], BF16, name="yw")

                        s_, d_, pp, q_ = yt(), yt(), yt(), yt()
                        o1, o2 = yt(), yt()
                        y = [yt() for _ in range(4)]
                        vv = nc.vector
                        vv.tensor_tensor(s_[:], e[1][:], e[2][:],
                                         op=ALU.add)
                        vv.tensor_tensor(pp[:], e[3][:], e[4][:],
                                         op=ALU.add)
                        vv.tensor_tensor(o1[:], s_[:], e[0][:], op=ALU.add)
                        vv.tensor_tensor(y[0][:], pp[:], o1[:], op=ALU.add)
                        vv.scalar_tensor_tensor(y[2][:], pp[:], 4.0, s_[:],
                                                op0=ALU.mult, op1=ALU.add)
                        vv.tensor_tensor(d_[:], e[1][:], e[2][:],
                                         op=ALU.subtract)
                        vv.tensor_tensor(q_[:], e[3][:], e[4][:],
                                         op=ALU.subtract)
                        vv.scalar_tensor_tensor(y[1][:], q_[:], 2.0, d_[:],
                                                op0=ALU.mult, op1=ALU.add)
                        vv.scalar_tensor_tensor(o2[:], q_[:], 8.0, d_[:],
                                                op0=ALU.mult, op1=ALU.add)
                        vv.tensor_tensor(y[3][:], o2[:], e[5][:],
                                         op=ALU.add)
                        for u_ in range(4):
                            nc.scalar.dma_start(
                                out[n, co * P:(co + 1) * P, u_,
                                    FB * b2:FB * b2 + FB],
                                y[u_][:],
                            )

            # ---- software-pipelined emission -----------------------------
            # DVE order: u0, T(0,ci0), U-rest, T(0,ci1), T(1), Y(0), T(2),
            # Y(1), T(3), Y(2), Y(3) — transforms always ahead of Y so the
            # PE never waits on V planes.
            emit_u0(0)
            emit_u0(1)
            vts0 = emit_transform(0, 0)
            emit_u_rest(0)
            emit_u_rest(1)
            vts1 = emit_transform(0, 1)
            vts = [vts0, vts1]
            evs: list = [None] * bp
            for n in range(bp):
                evs[n] = emit_mms(n, vts)
                if n + 1 < bp:
                    vts = [emit_transform(n + 1, 0),
                           emit_transform(n + 1, 1)]
                if n >= 1:
                    emit_y(n - 1, evs[n - 1])
            emit_y(bp - 1, evs[bp - 1])

    nc.compile()
    return nc


_NC_CACHE: dict[int, object] = {}


def _get_nc(bp: int = BP):
    if bp not in _NC_CACHE:
        _NC_CACHE[bp] = build(bp)
    return _NC_CACHE[bp]


def make_in_maps(x: np.ndarray, weight: np.ndarray, n_cores: int = N_CORES,
                 bp: int = BP):
    x = np.ascontiguousarray(x, dtype=np.float32)
    weight = np.ascontiguousarray(weight, dtype=np.float32)
    # d-plane marshalling (layout only): padded cols j=w+1 in 0..61,
    # d_s[r, t] = padded[r, 4t+s], flattened [58*15], stride-872 planes.
    padded = np.zeros((B, C, PR, 62), np.float32)
    padded[:, :, 1:H + 1, 1:W + 1] = x
    padded = padded.astype(ml_dtypes.bfloat16)
    xm = np.zeros((B, C, NP, DPL), ml_dtypes.bfloat16)
    for s in range(NP):
        xm[:, :, s, :FLAT] = padded[:, :, :, s::4][:, :, :, :PW15].reshape(
            B, C, FLAT)
    wp = np.ascontiguousarray(
        weight.reshape(O, C, KH * KW).transpose(2, 1, 0)
    ).astype(ml_dtypes.bfloat16)  # [t, i, o]
    return [
        {"x": xm[i * bp:(i + 1) * bp], "w": weight, "wp": wp}
        for i in range(n_cores)
    ]


def kernel(x: np.ndarray, weight: np.ndarray) -> np.ndarray:
    nc = _get_nc(BP)
    in_maps = make_in_maps(x, weight)
    res = run_bass_kernel_spmd(nc, in_maps, core_ids=list(range(N_CORES)))
    out = np.empty((B, O, H, W), dtype=np.float32)
    for i in range(N_CORES):
        od = res.results[i]["out"]  # [bp, O, 4, 840] bf16
        od = od.astype(np.float32).reshape(BP, O, 4, H, PW15)
        od = od.transpose(0, 1, 3, 4, 2).reshape(BP, O, H, PW15 * 4)
        out[i * BP:(i + 1) * BP] = od[:, :, :, :W]
    return out


# revision 13
# speedup vs baseline: 1.1644x; 1.1644x over previous
"""BinaryConv (XNOR-style binary-weight 3x3 conv) on 8 Trainium2 NeuronCores.

Full-input contract: kernel(x=[32,256,56,56] f32, weight=[256,256,3,3] f32)
-> [32,256,56,56] f32.

Strategy: data-parallel over batch (4 images/core), weight replicated.
Per core, a 1-D Winograd F(4,3) decomposition along W cuts the tensor-engine
MAC count 2x vs direct convolution (6 transformed positions per 4 output
columns instead of 12 tap-MACs):

  y = A'^T [ (G' s) . (B^T d) ]   per output row, with the 3 kh taps and the
                                  2 ci chunks accumulated in PSUM.

All math on device; host marshalling is layout/dtype only: x ships bf16,
de-interleaved into the six B^T operand planes d0..d5 (stride-4 phases of the
zero-padded rows, flattened 58x15 with a garbage 15th column) so every DVE op
and matmul rhs reads a contiguous, 4B-aligned window.  The weight transform
G'·sign(w) uses the row-rescaled dyadic G' (rows x[1,3,3,12,12,1]) so U is
exact in bf16; the matching column scales 1/c_p fold into the fp32
per-output-channel a=mean|w| (reduced on GPSIMD) applied at PSUM eviction on
ACT.  The input transform B^T and inverse transform A'^T run on DVE in bf16
via fused scalar_tensor_tensor ops; image n+1's input transform is emitted
ahead of image n's inverse transform so the DVE never stalls the PE.  The
output ships phase-major bf16 and is re-interleaved (and cast f32) on host.
"""

import ml_dtypes
import numpy as np

import concourse.mybir as mybir
import concourse.tile as tile
from concourse import bacc
from concourse.bass_utils import run_bass_kernel_spmd

F32 = mybir.dt.float32
BF16 = mybir.dt.bfloat16
ALU = mybir.AluOpType

N_CORES = 8
B, C, H, W = 32, 256, 56, 56
O, KH, KW = 256, 3, 3
BP = B // N_CORES            # images per core
P = 128                      # partitions
NCI = C // P                 # input-channel chunks
NCO = O // P                 # output-channel chunks
NP = 6                       # winograd positions (F(4,3): m+r-1 = 6)
TX = 14                      # output tiles along W (4 cols each)
PR = H + 2                   # padded rows (h -1..56)
PW15 = 15                    # tile columns incl. garbage col 14
DPL = PR * PW15 + 2          # 872: d-plane stride (870 valid + 2 pad)
FLAT = PR * PW15             # 870: flat transform window
OUTF = H * PW15              # 840: output flat length per (co, img)
FB = OUTF // 2               # 420: psum free size (2 batches)
KIN = C * KH * KW            # 2304 per-filter fan-in


def _u_off(p: int, kh: int, ci: int, co: int) -> int:
    return (((p * KH + kh) * NCI + ci) * NCO + co) * P


def build(bp: int = BP):
    nc = bacc.Bacc(
        "TRN2",
        target_bir_lowering=False,
        debug=False,
        enable_asserts=False,
        num_devices=N_CORES,
        enable_partition_id=False,
    )
    x_d = nc.dram_tensor("x", [bp, C, NP, DPL], BF16, kind="ExternalInput")
    w_d = nc.dram_tensor("w", [O, C, KH, KW], F32, kind="ExternalInput")
    wp_d = nc.dram_tensor("wp", [KH * KW, C, O], BF16, kind="ExternalInput")
    out_d = nc.dram_tensor("out", [bp, O, 4, OUTF], BF16,
                           kind="ExternalOutput")

    x = x_d.ap().rearrange("n (c p) s f -> n p c s f", p=P)
    w = w_d.ap().rearrange("o i kh kw -> o (i kh kw)")
    wp = wp_d.ap().rearrange("t (c p) o -> p c t o", p=P)
    out = out_d.ap()

    with tile.TileContext(nc) as tc:
        with (
            tc.tile_pool(name="const", bufs=1) as const_pool,
            tc.tile_pool(name="wstage", bufs=2) as wstage_pool,
            tc.tile_pool(name="xph", bufs=3) as xph_pool,
            tc.tile_pool(name="vt", bufs=4) as v_pool,
            tc.tile_pool(name="tmp", bufs=13) as tmp_pool,
            tc.tile_pool(name="ev", bufs=10) as e_pool,
            tc.tile_pool(name="yt", bufs=14) as y_pool,
            tc.tile_pool(name="psum", bufs=3, space="PSUM") as psum_pool,
            tc.tile_pool(name="warmps", bufs=1, space="PSUM") as warmps_pool,
        ):
            # ---- PE warmup: hold HAM at 2.4GHz through the input ramp ----
            warm_l = const_pool.tile([P, P], BF16)
            warm_r = const_pool.tile([P, 512], BF16)
            nc.gpsimd.memset(warm_l[:], 0.0)
            nc.gpsimd.memset(warm_r[:], 0.0)
            zbias = const_pool.tile([P, 1], F32)
            zscr = const_pool.tile([P, 1], F32)
            nc.gpsimd.memset(zbias[:], 0.0)
            warm_ps = warmps_pool.tile([P, 512], F32)
            for _ in range(14):
                nc.tensor.matmul(warm_ps[:], warm_l[:], warm_r[:],
                                 start=True, stop=True)
            # preload the Sign LUT on ACT before the weights arrive
            nc.scalar.sign(zscr[:], zbias[:], bias=zbias[:])

            # ---- critical-path input DMAs on the sync ring (FIFO) --------
            wpt = [const_pool.tile([P, KH * KW, O], BF16, name="wpt")
                   for _ in range(NCI)]
            nc.sync.dma_start(wpt[0][:], wp[:, 0])
            nc.sync.dma_start(wpt[1][:], wp[:, 1])
            xts: list[list] = [[None] * NCI for _ in range(bp)]

            def emit_x_dma(n, ci):
                xt = xph_pool.tile([P, NP, DPL], BF16, name="xt")
                nc.sync.dma_start(xt[:], x[n, :, ci])
                xts[n][ci] = xt

            emit_x_dma(0, 0)
            emit_x_dma(0, 1)
            wstages = [wstage_pool.tile([P, KIN], F32, name="ws")
                       for _ in range(NCO)]
            nc.sync.dma_start(wstages[0][:], w[0:P, :])
            nc.sync.dma_start(wstages[1][:], w[P:2 * P, :])
            for n in range(1, bp):
                emit_x_dma(n, 0)
                emit_x_dma(n, 1)

            # ---- signs (ACT); U tiles built on DVE, interleaved ----------
            st = [const_pool.tile([P, KH * KW, O], BF16, name="st")
                  for _ in range(NCI)]
            for ci in range(NCI):
                nc.scalar.sign(st[ci][:], wpt[ci][:], bias=zbias[:])

            upack = const_pool.tile([P, NP * KH * NCI * O], BF16)
            up = upack[:].rearrange("q (p kh ci o) -> q p kh ci o",
                                    p=NP, kh=KH, ci=NCI)

            def emit_u0(ci):
                sv = st[ci][:].rearrange("q (kh kw) o -> q kh kw o", kh=KH)
                nc.vector.tensor_scalar_mul(up[:, 0, :, ci], sv[:, :, 0],
                                            0.25)

            def emit_u_rest(ci):
                sv = st[ci][:].rearrange("q (kh kw) o -> q kh kw o", kh=KH)
                s0, s1, s2 = sv[:, :, 0], sv[:, :, 1], sv[:, :, 2]
                u = [up[:, p, :, ci] for p in range(NP)]
                vv = nc.vector
                t1 = const_pool.tile([P, KH, O], BF16, name="t1")
                s1h = const_pool.tile([P, KH, O], BF16, name="s1h")
                q_ = const_pool.tile([P, KH, O], BF16, name="q_")
                vv.tensor_tensor(t1[:], s0, s2, op=ALU.add)
                vv.tensor_scalar_mul(s1h[:], s1, 0.5)
                vv.scalar_tensor_tensor(u[1], t1[:], -0.5, s1h[:],
                                        op0=ALU.mult, op1=ALU.subtract)
                vv.tensor_tensor(u[2], u[1], s1, op=ALU.add)
                vv.scalar_tensor_tensor(q_[:], s2, 2.0, s1,
                                        op0=ALU.mult, op1=ALU.add)
                vv.scalar_tensor_tensor(u[3], s0, 0.5, q_[:],
                                        op0=ALU.mult, op1=ALU.add)
                vv.scalar_tensor_tensor(u[4], s1, -2.0, u[3],
                                        op0=ALU.mult, op1=ALU.add)
                vv.tensor_copy(u[5], s2)

            # ---- a = mean|w| with A'^T column scales, all on ACT ----------
            # av[c][:, co]: c=0 -> a, c=1 -> a/3, c=2 -> a/12  (fp32),
            # computed directly as accum_out of scaled |.| passes.
            av = [const_pool.tile([P, NCO], F32, name=f"av{c}")
                  for c in range(3)]
            scales = [1.0 / KIN, 1.0 / (3 * KIN), 1.0 / (12 * KIN)]
            wabs = wstage_pool.tile([P, KIN], BF16, name="wabs", bufs=1)
            for co in range(NCO):
                for c in range(3):
                    nc.scalar.activation(wabs[:], wstages[co][:],
                                         mybir.ActivationFunctionType.Abs,
                                         scale=scales[c],
                                         accum_out=av[c][:, co:co + 1])
            av_of_p = [av[0], av[1], av[1], av[2], av[2], av[0]]

            def emit_transform(n, ci):
                """Input transform: V[p] = B^T d, bf16 contiguous.

                tensor_tensor (DVE, 2x) + the unary scales on ACT (frees
                DVE cycles; scalar_tensor_tensor has no 2x uop so a fused
                form would run 1x and lose)."""
                xt = xts[n][ci]
                d = [xt[:, s, 0:FLAT] for s in range(NP)]
                vt = v_pool.tile([P, NP, FLAT], BF16, name="vt")
                v = [vt[:, p, :] for p in range(NP)]

                def tmp():
                    return tmp_pool.tile([P, FLAT], BF16, name="tw")

                i_, k_, g_, n_ = tmp(), tmp(), tmp(), tmp()
                ap_, bp_, am_, f_ = tmp(), tmp(), tmp(), tmp()
                k4, a4, am4, g2, g4 = tmp(), tmp(), tmp(), tmp(), tmp()
                vv = nc.vector
                vv.tensor_tensor(i_[:], d[4], d[2], op=ALU.subtract)
                vv.tensor_tensor(k_[:], d[0], d[2], op=ALU.subtract)
                vv.tensor_scalar_mul(k4[:], k_[:], 4.0)
                vv.tensor_tensor(v[0], k4[:], i_[:], op=ALU.add)
                vv.tensor_tensor(ap_[:], d[1], d[2], op=ALU.add)
                vv.tensor_tensor(bp_[:], d[3], d[4], op=ALU.add)
                vv.tensor_scalar_mul(a4[:], ap_[:], -4.0)
                vv.tensor_tensor(v[1], a4[:], bp_[:], op=ALU.add)
                vv.tensor_tensor(am_[:], d[1], d[2], op=ALU.subtract)
                vv.tensor_tensor(f_[:], d[3], d[4], op=ALU.subtract)
                vv.tensor_scalar_mul(am4[:], am_[:], 4.0)
                vv.tensor_tensor(v[2], am4[:], f_[:], op=ALU.subtract)
                vv.tensor_tensor(g_[:], d[3], d[1], op=ALU.subtract)
                vv.tensor_scalar_mul(g2[:], g_[:], 2.0)
                vv.tensor_tensor(v[3], g2[:], i_[:], op=ALU.add)
                vv.tensor_tensor(v[4], i_[:], g2[:], op=ALU.subtract)
                vv.tensor_tensor(n_[:], d[5], d[3], op=ALU.subtract)
                vv.tensor_scalar_mul(g4[:], g_[:], -4.0)
                vv.tensor_tensor(v[5], g4[:], n_[:], op=ALU.add)
                return vt

            def emit_mms(n, vts):
                """Matmuls + scaled PSUM eviction for image n.

                Each (co, p) uses one 2-bank PSUM pair-tile [P, 1024] with
                the two output batches at offsets 0 and 512 (each within a
                bank), so a single ACT eviction drains both.  The two MMs
                sharing a stationary U tile are adjacent (halves exposed
                LDWEIGHTS).  Returns E planes ev[co][p] as [P, 1024] tiles
                with valid runs [0:FB] and [512:512+FB]."""
                ev = [[None] * NP for _ in range(NCO)]
                for co in range(NCO):
                    for p in range(NP):
                        ps = psum_pool.tile([P, 1024], F32, name="ps")
                        for ci in range(NCI):
                            for kh in range(KH):
                                off = _u_off(p, kh, ci, co)
                                first = ci == 0 and kh == 0
                                last = ci == NCI - 1 and kh == KH - 1
                                for b2 in range(2):
                                    rhs = vts[ci][:, p,
                                                  PW15 * kh + FB * b2:
                                                  PW15 * kh + FB * b2 + FB]
                                    nc.tensor.matmul(
                                        ps[:, 512 * b2:512 * b2 + FB],
                                        upack[:, off:off + P],
                                        rhs, start=first, stop=last,
                                    )
                        et = e_pool.tile([P, 1024], BF16, name="et")
                        nc.scalar.mul(et[:, 0:512 + FB],
                                      ps[:, 0:512 + FB],
                                      av_of_p[p][:, co:co + 1])
                        ev[co][p] = et
                return ev

            def emit_y(n, ev):
                """Inverse transform A'^T: tensor_tensor on DVE (bf16 2x),
                unary scales on ACT, both output batches fused per op.
                Output DMA triggers ride the sync ring."""
                for co in range(NCO):
                    # 2-run views [P, 2, FB] over the E pair tiles
                    e = [ev[co][p][:].rearrange("q (b g) -> q b g", b=2)
                         [:, :, 0:FB] for p in range(NP)]

                    def yt():
                        t = y_pool.tile([P, OUTF], BF16, name="yw")
                        return t[:].rearrange("q (b f) -> q b f", b=2), t

                    s_, _ts = yt()
                    d_, _td = yt()
                    pp, _tp = yt()
                    q_, _tq = yt()
                    o1, _ = yt()
                    q2, _ = yt()
                    p4, _ = yt()
                    q8, _ = yt()
                    o2, _ = yt()
                    yv, yfull = [], []
                    for _u in range(4):
                        v2, t = yt()
                        yv.append(v2)
                        yfull.append(t)
                    vv = nc.vector
                    sc = nc.scalar
                    vv.tensor_tensor(s_, e[1], e[2], op=ALU.add)
                    vv.tensor_tensor(pp, e[3], e[4], op=ALU.add)
                    vv.tensor_tensor(o1, s_, e[0], op=ALU.add)
                    vv.tensor_tensor(yv[0], pp, o1, op=ALU.add)
                    sc.mul(p4, pp, 4.0)
                    vv.tensor_tensor(yv[2], p4, s_, op=ALU.add)
                    vv.tensor_tensor(d_, e[1], e[2], op=ALU.subtract)
                    vv.tensor_tensor(q_, e[3], e[4], op=ALU.subtract)
                    sc.mul(q2, q_, 2.0)
                    vv.tensor_tensor(yv[1], q2, d_, op=ALU.add)
                    sc.mul(q8, q_, 8.0)
                    vv.tensor_tensor(o2, q8, d_, op=ALU.add)
                    vv.tensor_tensor(yv[3], o2, e[5], op=ALU.add)
                    for u_ in range(4):
                        nc.sync.dma_start(
                            out[n, co * P:(co + 1) * P, u_, :],
                            yfull[u_][:],
                        )

            # ---- software-pipelined emission -----------------------------
            # DVE order: u0, T(0,ci0), U-rest, T(0,ci1), T(1), Y(0), T(2),
            # Y(1), T(3), Y(2), Y(3) — transforms always ahead of Y so the
            # PE never waits on V planes.
            emit_u0(0)
            emit_u0(1)
            vts0 = emit_transform(0, 0)
            emit_u_rest(0)
            emit_u_rest(1)
            vts1 = emit_transform(0, 1)
            vts = [vts0, vts1]
            evs: list = [None] * bp
            for n in range(bp):
                if n >= 1:
                    emit_y(n - 1, evs[n - 1])
                evs[n] = emit_mms(n, vts)
                if n + 1 < bp:
                    vts = [emit_transform(n + 1, 0),
                           emit_transform(n + 1, 1)]
            emit_y(bp - 1, evs[bp - 1])

    nc.compile()
    return nc


_NC_CACHE: dict[int, object] = {}


def _get_nc(bp: int = BP):
    if bp not in _NC_CACHE:
        _NC_CACHE[bp] = build(bp)
    return _NC_CACHE[bp]


def make_in_maps(x: np.ndarray, weight: np.ndarray, n_cores: int = N_CORES,
                 bp: int = BP):
    x = np.ascontiguousarray(x, dtype=np.float32)
    weight = np.ascontiguousarray(weight, dtype=np.float32)
    # d-plane marshalling (layout only): padded cols j=w+1 in 0..61,
    # d_s[r, t] = padded[r, 4t+s], flattened [58*15], stride-872 planes.
    padded = np.zeros((B, C, PR, 62), np.float32)
    padded[:, :, 1:H + 1, 1:W + 1] = x
    padded = padded.astype(ml_dtypes.bfloat16)
    xm = np.zeros((B, C, NP, DPL), ml_dtypes.bfloat16)
    for s in range(NP):
        xm[:, :, s, :FLAT] = padded[:, :, :, s::4][:, :, :, :PW15].reshape(
            B, C, FLAT)
    wp = np.ascontiguousarray(
        weight.reshape(O, C, KH * KW).transpose(2, 1, 0)
    ).astype(ml_dtypes.bfloat16)  # [t, i, o]
    return [
        {"x": xm[i * bp:(i + 1) * bp], "w": weight, "wp": wp}
        for i in range(n_cores)
    ]


def kernel(x: np.ndarray, weight: np.ndarray) -> np.ndarray:
    nc = _get_nc(BP)
    in_maps = make_in_maps(x, weight)
    res = run_bass_kernel_spmd(nc, in_maps, core_ids=list(range(N_CORES)))
    out = np.empty((B, O, H, W), dtype=np.float32)
    for i in range(N_CORES):
        od = res.results[i]["out"]  # [bp, O, 4, 840] bf16
        od = od.astype(np.float32).reshape(BP, O, 4, H, PW15)
        od = od.transpose(0, 1, 3, 4, 2).reshape(BP, O, H, PW15 * 4)
        out[i * BP:(i + 1) * BP] = od[:, :, :, :W]
    return out
